# revision 13
# baseline (speedup 1.0000x reference)
"""Bahdanau additive self-attention (causal) on 8 Trainium2 NeuronCores.

reference math (B=4, T=512, H=256):
  q = x @ W1.T ; k = x @ W2.T
  scores[b,t,s] = sum_h v[h] * tanh(q[b,t,h] + k[b,s,h])   (causal: s <= t)
  weights = softmax(scores, axis=-1) ; context = weights @ x
returns (context, weights)

Sharding: 2 cores per batch. Each core handles 4 half-blocks of 64 query
rows chosen so every core's causal s-extents are exactly {128,256,384,512}
-> one identical SPMD program; only the per-core host-side gather/scatter
and the additive causal-mask input differ per core.

Per-core kernel layout: tanh input is built in (h=partition, s=free)
layout so the q[t]+k[s] add is a per-partition tensor_scalar on DVE and
the sum_h v*tanh() reduction is a matmul with a column-selector matrix
built from v (accumulating score rows into PSUM via 32-column tiling).
"""

import numpy as np
import ml_dtypes

B, T, H = 4, 512, 256
NCORES = 8
G = 32  # t-rows per fat qk tile

# per-parity half-blocks: (t0, padded causal extent S)
HBS = {
    0: [(0, 128), (128, 256), (320, 384), (448, 512)],
    1: [(64, 128), (192, 256), (256, 384), (384, 512)],
}
EXTENTS = [128, 256, 384, 512]

_CACHE = {}


def _build_nc():
    from contextlib import ExitStack

    import concourse.bass as bass
    import concourse.tile as tile
    from concourse import bacc, mybir

    f32 = mybir.dt.float32
    bf16 = mybir.dt.bfloat16
    AF = mybir.ActivationFunctionType
    AX = mybir.AxisListType
    ALU = mybir.AluOpType
    PSUM = bass.MemorySpace.PSUM

    nc = bacc.Bacc("TRN2", target_bir_lowering=False, debug=False, num_devices=NCORES)

    xqt = nc.dram_tensor("xqt", [H, 256], f32, kind="ExternalInput").ap()
    xkt = nc.dram_tensor("xkt", [H, T], f32, kind="ExternalInput").ap()
    xs = nc.dram_tensor("xs", [T, H], f32, kind="ExternalInput").ap()
    w1t = nc.dram_tensor("w1t", [H, H], f32, kind="ExternalInput").ap()
    w2t = nc.dram_tensor("w2t", [H, H], f32, kind="ExternalInput").ap()
    vsel = nc.dram_tensor("vsel", [128, 2 * 32 * 32], bf16, kind="ExternalInput").ap()
    msk = nc.dram_tensor("msk", [4 * 64, T], f32, kind="ExternalInput").ap()
    ident = nc.dram_tensor("ident", [64, 64], f32, kind="ExternalInput").ap()
    wts = nc.dram_tensor("wts", [4 * 64, T], f32, kind="ExternalOutput").ap()
    ctx = nc.dram_tensor("ctx", [256, H], f32, kind="ExternalOutput").ap()

    with tile.TileContext(nc) as tc, ExitStack() as es:
        P1 = es.enter_context(tc.tile_pool(name="persist", bufs=1))
        Pqk = es.enter_context(tc.tile_pool(name="qk", bufs=2))
        Pth = es.enter_context(tc.tile_pool(name="th", bufs=2))
        Psm = es.enter_context(tc.tile_pool(name="sm", bufs=2))
        Pset = es.enter_context(tc.tile_pool(name="pset", bufs=2, space=PSUM))
        Psc = es.enter_context(tc.tile_pool(name="psc", bufs=2, space=PSUM))
        Pwt = es.enter_context(tc.tile_pool(name="pwt", bufs=2, space=PSUM))
        Pcx = es.enter_context(tc.tile_pool(name="pcx", bufs=1, space=PSUM))

        # ---- persistent loads: split into ~64KB pieces across many DMA
        #      queues (single-queue BW is ~25GB/s), critical path first ----
        def load(dram_ap, shape, dtype, tag, eng=None, pieces=1):
            t_ = P1.tile(shape, dtype, tag=tag)
            w = shape[1] // pieces
            for pc in range(pieces):
                (eng or nc.sync).dma_start(
                    t_[:, pc * w:(pc + 1) * w],
                    dram_ap[:, pc * w:(pc + 1) * w])
            return t_

        # slots 0/1 (processed in the first wave) only need k-cols [0:256]
        # and q-cols [0:128]; load those pieces first and build small early
        # KT/QT tiles so tanh work starts sooner.
        xktA = [load(xkt[k * 128:(k + 1) * 128, 0:256], [128, 256], f32,
                     f"xktA{k}", pieces=2) for k in range(2)]
        w2t_sb = [load(w2t[k * 128:(k + 1) * 128, :], [128, H], f32, f"w2t{k}")
                  for k in range(2)]
        xqtA = [load(xqt[k * 128:(k + 1) * 128, 0:128], [128, 128], f32,
                     f"xqtA{k}") for k in range(2)]
        w1t_sb = [load(w1t[k * 128:(k + 1) * 128, :], [128, H], f32, f"w1t{k}")
                  for k in range(2)]
        xkt_sb = [load(xkt[k * 128:(k + 1) * 128, :], [128, T], f32, f"xkt{k}",
                       eng=nc.gpsimd, pieces=2) for k in range(2)]
        xqt_sb = [load(xqt[k * 128:(k + 1) * 128, :], [128, 256], f32,
                       f"xqt{k}", eng=nc.gpsimd) for k in range(2)]
        vsel_sb = load(vsel[:, :], [128, 2 * 32 * 32], bf16, "vsel", pieces=2)
        ident_sb = load(ident[:, :], [64, 64], f32, "ident", eng=nc.gpsimd)
        # ACT table warmup
        warm = Psm.tile([1, 1], f32, tag="warm")
        nc.scalar.activation(warm[:], ident_sb[0:1, 0:1], AF.Tanh)
        xs_sb = [load(xs[s * 128:(s + 1) * 128, :], [128, H], f32, f"xs{s}",
                      eng=nc.gpsimd) for s in range(4)]
        msk_sb = [load(msk[i * 64:(i + 1) * 64, :], [64, T], f32, f"msk{i}",
                       eng=nc.gpsimd) for i in range(4)]

        # ---- KT / QT:   kt[h, s] = k[s, h],  qt[h, t] = q[t_rows[t], h] ----
        kt_early, qt_early = [None, None], [None, None]
        for m in range(2):
            ktp = Pset.tile([128, 256], f32, tag="setup")
            for k in range(2):
                nc.tensor.matmul(ktp[:], w2t_sb[k][:, m * 128:(m + 1) * 128],
                                 xktA[k][:], start=(k == 0), stop=(k == 1))
            k_ = P1.tile([128, 256], bf16, tag=f"kte{m}")
            nc.vector.tensor_copy(k_[:], ktp[:])
            kt_early[m] = k_
            qtp = Pset.tile([128, 128], f32, tag="setup")
            for k in range(2):
                nc.tensor.matmul(qtp[:], w1t_sb[k][:, m * 128:(m + 1) * 128],
                                 xqtA[k][:], start=(k == 0), stop=(k == 1))
            q_ = P1.tile([128, 128], f32, tag=f"qte{m}")
            nc.vector.tensor_copy(q_[:], qtp[:])
            qt_early[m] = q_
        qt_sb, kt_sb = [None, None], [None, None]
        for m in range(2):
            ktp = Pset.tile([128, T], f32, tag="setup")
            for k in range(2):
                nc.tensor.matmul(ktp[:], w2t_sb[k][:, m * 128:(m + 1) * 128],
                                 xkt_sb[k][:], start=(k == 0), stop=(k == 1))
            k_ = P1.tile([128, T], bf16, tag=f"kt{m}")
            nc.vector.tensor_copy(k_[:], ktp[:])
            kt_sb[m] = k_
            qtp = Pset.tile([128, 256], f32, tag="setup")
            for k in range(2):
                nc.tensor.matmul(qtp[:], w1t_sb[k][:, m * 128:(m + 1) * 128],
                                 xqt_sb[k][:], start=(k == 0), stop=(k == 1))
            q_ = P1.tile([128, 256], f32, tag=f"qt{m}")
            nc.vector.tensor_copy(q_[:], qtp[:])
            qt_sb[m] = q_

        # ---- main ----
        def preamble(hb):
            S = EXTENTS[hb]
            sc_psum = Psc.tile([64, S], f32, tag="scores")
            # ragged extents leave [E_r, S) unwritten by the MMs; memset so
            # the additive causal mask lands on zeros, not stale PSUM
            nc.vector.memset(sc_psum[:], 0.0)
            return sc_psum

        def unit(hb, g, c, sc_psum):
            """One (group, h-chunk): 32 q+k adds, one tanh, 32 score MMs.
            Group g covers 16 t-rows from each 32-row strip so the MM sweep
            can alternate col-groups (LDWEIGHTS of one strip overlaps the
            in-flight MATMUL of the other)."""
            S = EXTENTS[hb]
            kt_use = kt_early if hb in (0, 1) else kt_sb
            qt_use = qt_early if hb in (0, 1) else qt_sb
            tls = [g * 16 + (i % 16) + 32 * (i // 16) for i in range(G)]
            ext = [S - 63 + tl for tl in tls]  # exact causal extent
            offs = [0]
            for e in ext:
                offs.append(offs[-1] + e)
            fd = offs[-1]
            qk = Pqk.tile([128, fd], bf16, tag="qk")
            for i in range(G):
                tcol = hb * 64 + tls[i]
                nc.vector.tensor_scalar_add(
                    qk[:, offs[i]:offs[i] + ext[i]],
                    kt_use[c][:, :ext[i]],
                    qt_use[c][:, tcol:tcol + 1])
            th = Pth.tile([128, fd], bf16, tag="th")
            nc.scalar.activation(th[:], qk[:], AF.Tanh)
            for ii in range(16):
                for half in (0, 1):
                    i = ii + 16 * half
                    tl = tls[i]
                    j, col = tl // 32, tl % 32
                    lo = (c * 32 + col) * 32
                    # no start=True anywhere: the DVE memset is the zero
                    # base and every MM accumulates (first touch of a
                    # pending-zero byte overwrites with its product = 0+x)
                    nc.tensor.matmul(
                        sc_psum[32 * j:32 * j + 32, 0:ext[i]],
                        vsel_sb[:, lo:lo + 32],
                        th[:, offs[i]:offs[i] + ext[i]],
                        start=False,
                        stop=(g == 1 and c == 1 and ii == 15),
                        tile_position=(0, 32 * j),
                        skip_group_check=True)

        def postamble(hb, sc_psum):
            S = EXTENTS[hb]
            # mask + softmax
            nc.vector.tensor_add(sc_psum[:], sc_psum[:], msk_sb[hb][:, :S])
            nmax = Psm.tile([64, 1], f32, tag="nmax")
            nc.vector.tensor_reduce(nmax[:], sc_psum[:], axis=AX.X,
                                    op=ALU.max, negate=True)
            e_sb = Psm.tile([64, S], f32, tag="esb")
            rsum = Psm.tile([64, 1], f32, tag="rsum")
            nc.scalar.activation(e_sb[:], sc_psum[:], AF.Exp, bias=nmax[:],
                                 accum_out=rsum[:])
            rinv = Psm.tile([64, 1], f32, tag="rinv")
            nc.vector.reciprocal(rinv[:], rsum[:])
            w_sb = Psm.tile([64, S], f32, tag="wsb")
            nc.vector.tensor_scalar_mul(w_sb[:], e_sb[:], rinv[:])
            nc.sync.dma_start(wts[hb * 64:(hb + 1) * 64, :S], w_sb[:])

            # context = weights @ x  (transpose weights chunkwise on PE)
            cxp = Pcx.tile([64, H], f32, tag="cxp")
            nchunk = S // 128
            for sc in range(nchunk):
                wtp = Pwt.tile([128, 64], f32, tag="wtp")
                nc.tensor.transpose(wtp[:], w_sb[:, sc * 128:(sc + 1) * 128],
                                    ident_sb[:])
                wt_s = Psm.tile([128, 64], f32, tag="wtsb")
                nc.vector.tensor_copy(wt_s[:], wtp[:])
                nc.tensor.matmul(cxp[:], wt_s[:], xs_sb[sc][:],
                                 start=(sc == 0), stop=(sc == nchunk - 1))
            cx_sb = Psm.tile([64, H], f32, tag="cxsb")
            nc.vector.tensor_copy(cx_sb[:], cxp[:])
            nc.sync.dma_start(ctx[hb * 64:(hb + 1) * 64, :], cx_sb[:])

        # interleave units of slot pairs so the big slot's dense tanh work
        # fills the small slot's DVE-bound gaps; end on the cheapest slot
        # (0) for a short tail. Early-KT slots (1, 0) lead each pair so
        # work starts before the full KT/QT tiles are ready.
        for first, second in ((1, 2), (3, 0)):
            scs = {first: preamble(first), second: preamble(second)}
            order = []
            for g in range(2):
                for c in range(2):
                    order += [(first, g, c), (second, g, c)]
            # put the pair-closing units so 'second' finishes last
            for hb, g, c in order:
                unit(hb, g, c, scs[hb])
                if hb == first and g == 1 and c == 1:
                    postamble(first, scs[first])
            postamble(second, scs[second])

    nc.compile()
    return nc


def get_nc():
    if "nc" not in _CACHE:
        _CACHE["nc"] = _build_nc()
    return _CACHE["nc"]


def _make_vsel(v):
    # vsel[k, (c*32+col)*32 + m] = v[c*128+k] if m == col else 0
    out = np.zeros((128, 2 * 32 * 32), np.float32)
    for c in range(2):
        for col in range(32):
            out[:, (c * 32 + col) * 32 + col] = v[c * 128:(c + 1) * 128]
    return out.astype(ml_dtypes.bfloat16)


def _make_mask(parity):
    m = np.full((4 * 64, T), -1e30, np.float32)
    for i, (t0, _) in enumerate(HBS[parity]):
        for r in range(64):
            m[i * 64 + r, :t0 + r + 1] = 0.0
    return m


def make_in_maps(rnn_outputs, W1, W2, v):
    X = np.asarray(rnn_outputs, np.float32)
    W1 = np.asarray(W1, np.float32)
    W2 = np.asarray(W2, np.float32)
    v = np.asarray(v, np.float32)
    vsel_np = _make_vsel(v)
    ident_np = np.eye(64, dtype=np.float32)
    masks = {p: _make_mask(p) for p in (0, 1)}
    w1t_np = np.ascontiguousarray(W1.T)
    w2t_np = np.ascontiguousarray(W2.T)
    in_maps = []
    for core in range(NCORES):
        b, p = core // 2, core % 2
        t_rows = np.concatenate([np.arange(t0, t0 + 64) for t0, _ in HBS[p]])
        xb = np.ascontiguousarray(X[b])
        in_maps.append({
            "xqt": np.ascontiguousarray(xb[t_rows].T),
            "xkt": np.ascontiguousarray(xb.T),
            "xs": xb,
            "w1t": w1t_np,
            "w2t": w2t_np,
            "vsel": vsel_np,
            "msk": masks[p],
            "ident": ident_np,
        })
    return in_maps


def assemble(results):
    context = np.zeros((B, T, H), np.float32)
    weights = np.zeros((B, T, T), np.float32)
    for core in range(NCORES):
        b, p = core // 2, core % 2
        w_ = np.asarray(results[core]["wts"]).reshape(4, 64, T)
        c_ = np.asarray(results[core]["ctx"]).reshape(4, 64, H)
        for i, (t0, S) in enumerate(HBS[p]):
            weights[b, t0:t0 + 64, :S] = w_[i, :, :S]
            context[b, t0:t0 + 64] = c_[i]
    return context, weights


def kernel(rnn_outputs, W1, W2, v):
    from concourse.bass_utils import run_bass_kernel_spmd

    nc = get_nc()
    in_maps = make_in_maps(rnn_outputs, W1, W2, v)
    res = run_bass_kernel_spmd(nc, in_maps, list(range(NCORES))).results
    return assemble(res)


# revision 14
# speedup vs baseline: 1.1227x; 1.1227x over previous
"""Bahdanau additive self-attention (causal) on 8 Trainium2 NeuronCores.

reference math (B=4, T=512, H=256):
  q = x @ W1.T ; k = x @ W2.T
  scores[b,t,s] = sum_h v[h] * tanh(q[b,t,h] + k[b,s,h])   (causal: s <= t)
  weights = softmax(scores, axis=-1) ; context = weights @ x
returns (context, weights)

Sharding: 2 cores per batch. Each core handles 4 half-blocks of 64 query
rows chosen so every core's causal s-extents are exactly {128,256,384,512}
-> one identical SPMD program; only the per-core host-side gather/scatter
and the additive causal-mask input differ per core.

Per-core kernel layout: tanh input is built in (h=partition, s=free)
layout so the q[t]+k[s] add is a per-partition tensor_scalar on DVE and
the sum_h v*tanh() reduction is a matmul with a column-selector matrix
built from v (accumulating score rows into PSUM via 32-column tiling).
"""

import numpy as np
import ml_dtypes

B, T, H = 4, 512, 256
NCORES = 8
G = 16  # t-rows per fat qk tile

# per-parity half-blocks: (t0, padded causal extent S)
HBS = {
    0: [(0, 128), (128, 256), (320, 384), (448, 512)],
    1: [(64, 128), (192, 256), (256, 384), (384, 512)],
}
EXTENTS = [128, 256, 384, 512]

_CACHE = {}


def _build_nc():
    from contextlib import ExitStack

    import concourse.bass as bass
    import concourse.tile as tile
    from concourse import bacc, mybir

    f32 = mybir.dt.float32
    bf16 = mybir.dt.bfloat16
    AF = mybir.ActivationFunctionType
    AX = mybir.AxisListType
    ALU = mybir.AluOpType
    PSUM = bass.MemorySpace.PSUM

    nc = bacc.Bacc("TRN2", target_bir_lowering=False, debug=False, num_devices=NCORES)

    xqt = nc.dram_tensor("xqt", [H, 256], f32, kind="ExternalInput").ap()
    xkt = nc.dram_tensor("xkt", [H, T], f32, kind="ExternalInput").ap()
    xs = nc.dram_tensor("xs", [T, H], f32, kind="ExternalInput").ap()
    w1t = nc.dram_tensor("w1t", [H, H], f32, kind="ExternalInput").ap()
    w2t = nc.dram_tensor("w2t", [H, H], f32, kind="ExternalInput").ap()
    vsel = nc.dram_tensor("vsel", [128, 2 * 32 * 32], bf16, kind="ExternalInput").ap()
    msk = nc.dram_tensor("msk", [4 * 64, T], f32, kind="ExternalInput").ap()
    ident = nc.dram_tensor("ident", [64, 64], f32, kind="ExternalInput").ap()
    wts = nc.dram_tensor("wts", [4 * 64, T], f32, kind="ExternalOutput").ap()
    ctx = nc.dram_tensor("ctx", [256, H], f32, kind="ExternalOutput").ap()

    with tile.TileContext(nc) as tc, ExitStack() as es:
        P1 = es.enter_context(tc.tile_pool(name="persist", bufs=1))
        Pqk = es.enter_context(tc.tile_pool(name="qk", bufs=4))
        Pth = es.enter_context(tc.tile_pool(name="th", bufs=4))
        Psm = es.enter_context(tc.tile_pool(name="sm", bufs=2))
        Pset = es.enter_context(tc.tile_pool(name="pset", bufs=2, space=PSUM))
        Psc = es.enter_context(tc.tile_pool(name="psc", bufs=2, space=PSUM))
        Pwt = es.enter_context(tc.tile_pool(name="pwt", bufs=2, space=PSUM))
        Pcx = es.enter_context(tc.tile_pool(name="pcx", bufs=1, space=PSUM))

        # ---- persistent loads: split into ~64KB pieces across many DMA
        #      queues (single-queue BW is ~25GB/s), critical path first ----
        def load(dram_ap, shape, dtype, tag, eng=None, pieces=1):
            t_ = P1.tile(shape, dtype, tag=tag)
            w = shape[1] // pieces
            for pc in range(pieces):
                (eng or nc.sync).dma_start(
                    t_[:, pc * w:(pc + 1) * w],
                    dram_ap[:, pc * w:(pc + 1) * w])
            return t_

        # slots 0/1 (processed in the first wave) only need k-cols [0:256]
        # and q-cols [0:128]; load those pieces first and build small early
        # KT/QT tiles so tanh work starts sooner.
        xktA = [load(xkt[k * 128:(k + 1) * 128, 0:256], [128, 256], f32,
                     f"xktA{k}", pieces=2) for k in range(2)]
        w2t_sb = [load(w2t[k * 128:(k + 1) * 128, :], [128, H], f32, f"w2t{k}")
                  for k in range(2)]
        xqtA = [load(xqt[k * 128:(k + 1) * 128, 0:128], [128, 128], f32,
                     f"xqtA{k}") for k in range(2)]
        w1t_sb = [load(w1t[k * 128:(k + 1) * 128, :], [128, H], f32, f"w1t{k}")
                  for k in range(2)]
        xkt_sb = [load(xkt[k * 128:(k + 1) * 128, :], [128, T], f32, f"xkt{k}",
                       eng=nc.gpsimd, pieces=2) for k in range(2)]
        xqt_sb = [load(xqt[k * 128:(k + 1) * 128, :], [128, 256], f32,
                       f"xqt{k}", eng=nc.gpsimd) for k in range(2)]
        vsel_sb = load(vsel[:, :], [128, 2 * 32 * 32], bf16, "vsel", pieces=2)
        ident_sb = load(ident[:, :], [64, 64], f32, "ident", eng=nc.gpsimd)
        # ACT table warmup
        warm = Psm.tile([1, 1], f32, tag="warm")
        nc.scalar.activation(warm[:], ident_sb[0:1, 0:1], AF.Tanh)
        xs_sb = [load(xs[s * 128:(s + 1) * 128, :], [128, H], f32, f"xs{s}",
                      eng=nc.gpsimd) for s in range(4)]
        msk_sb = [load(msk[i * 64:(i + 1) * 64, :], [64, T], f32, f"msk{i}",
                       eng=nc.gpsimd) for i in range(4)]

        # ---- KT / QT:   kt[h, s] = k[s, h],  qt[h, t] = q[t_rows[t], h] ----
        kt_early, qt_early = [None, None], [None, None]
        for m in range(2):
            ktp = Pset.tile([128, 256], f32, tag="setup")
            for k in range(2):
                nc.tensor.matmul(ktp[:], w2t_sb[k][:, m * 128:(m + 1) * 128],
                                 xktA[k][:], start=(k == 0), stop=(k == 1))
            k_ = P1.tile([128, 256], bf16, tag=f"kte{m}")
            nc.vector.tensor_copy(k_[:], ktp[:])
            kt_early[m] = k_
            qtp = Pset.tile([128, 128], f32, tag="setup")
            for k in range(2):
                nc.tensor.matmul(qtp[:], w1t_sb[k][:, m * 128:(m + 1) * 128],
                                 xqtA[k][:], start=(k == 0), stop=(k == 1))
            q_ = P1.tile([128, 128], f32, tag=f"qte{m}")
            nc.vector.tensor_copy(q_[:], qtp[:])
            qt_early[m] = q_
        qt_sb, kt_sb = [None, None], [None, None]
        for m in range(2):
            ktp = Pset.tile([128, T], f32, tag="setup")
            for k in range(2):
                nc.tensor.matmul(ktp[:], w2t_sb[k][:, m * 128:(m + 1) * 128],
                                 xkt_sb[k][:], start=(k == 0), stop=(k == 1))
            k_ = P1.tile([128, T], bf16, tag=f"kt{m}")
            nc.vector.tensor_copy(k_[:], ktp[:])
            kt_sb[m] = k_
            qtp = Pset.tile([128, 256], f32, tag="setup")
            for k in range(2):
                nc.tensor.matmul(qtp[:], w1t_sb[k][:, m * 128:(m + 1) * 128],
                                 xqt_sb[k][:], start=(k == 0), stop=(k == 1))
            q_ = P1.tile([128, 256], f32, tag=f"qt{m}")
            nc.vector.tensor_copy(q_[:], qtp[:])
            qt_sb[m] = q_

        # ---- main ----
        def preamble(hb):
            S = EXTENTS[hb]
            sc_psum = Psc.tile([64, S], f32, tag="scores")
            # ragged extents leave [E_r, S) unwritten by the MMs; memset so
            # the additive causal mask lands on zeros, not stale PSUM
            nc.vector.memset(sc_psum[:], 0.0)
            return sc_psum

        def unit(hb, g, c, sc_psum):
            """One (group, h-chunk): 32 q+k adds, one tanh, 32 score MMs.
            Group g covers 16 t-rows from each 32-row strip so the MM sweep
            can alternate col-groups (LDWEIGHTS of one strip overlaps the
            in-flight MATMUL of the other)."""
            S = EXTENTS[hb]
            kt_use = kt_early if hb in (0, 1) else kt_sb
            qt_use = qt_early if hb in (0, 1) else qt_sb
            half_g = G // 2
            tls = [g * half_g + (i % half_g) + 32 * (i // half_g)
                   for i in range(G)]
            ext = [S - 63 + tl for tl in tls]  # exact causal extent
            offs = [0]
            for e in ext:
                offs.append(offs[-1] + e)
            fd = offs[-1]
            qk = Pqk.tile([128, fd], bf16, tag="qk")
            for i in range(G):
                tcol = hb * 64 + tls[i]
                nc.vector.tensor_scalar_add(
                    qk[:, offs[i]:offs[i] + ext[i]],
                    kt_use[c][:, :ext[i]],
                    qt_use[c][:, tcol:tcol + 1])
            th = Pth.tile([128, fd], bf16, tag="th")
            nc.scalar.activation(th[:], qk[:], AF.Tanh)
            n_g = 64 // G
            for ii in range(G // 2):
                for half in (0, 1):
                    i = ii + (G // 2) * half
                    tl = tls[i]
                    j, col = tl // 32, tl % 32
                    lo = (c * 32 + col) * 32
                    # no start=True anywhere: the DVE memset is the zero
                    # base and every MM accumulates (first touch of a
                    # pending-zero byte overwrites with its product = 0+x)
                    nc.tensor.matmul(
                        sc_psum[32 * j:32 * j + 32, 0:ext[i]],
                        vsel_sb[:, lo:lo + 32],
                        th[:, offs[i]:offs[i] + ext[i]],
                        start=False,
                        stop=(g == n_g - 1 and c == 1 and ii == G // 2 - 1),
                        tile_position=(0, 32 * j),
                        skip_group_check=True)

        def postamble(hb, sc_psum):
            S = EXTENTS[hb]
            # mask + softmax
            nc.vector.tensor_add(sc_psum[:], sc_psum[:], msk_sb[hb][:, :S])
            nmax = Psm.tile([64, 1], f32, tag="nmax")
            nc.vector.tensor_reduce(nmax[:], sc_psum[:], axis=AX.X,
                                    op=ALU.max, negate=True)
            e_sb = Psm.tile([64, S], f32, tag="esb")
            rsum = Psm.tile([64, 1], f32, tag="rsum")
            nc.scalar.activation(e_sb[:], sc_psum[:], AF.Exp, bias=nmax[:],
                                 accum_out=rsum[:])
            rinv = Psm.tile([64, 1], f32, tag="rinv")
            nc.vector.reciprocal(rinv[:], rsum[:])
            w_sb = Psm.tile([64, S], f32, tag="wsb")
            nc.vector.tensor_scalar_mul(w_sb[:], e_sb[:], rinv[:])
            nc.sync.dma_start(wts[hb * 64:(hb + 1) * 64, :S], w_sb[:])

            # context = weights @ x  (transpose weights chunkwise on PE)
            cxp = Pcx.tile([64, H], f32, tag="cxp")
            nchunk = S // 128
            for sc in range(nchunk):
                wtp = Pwt.tile([128, 64], f32, tag="wtp")
                nc.tensor.transpose(wtp[:], w_sb[:, sc * 128:(sc + 1) * 128],
                                    ident_sb[:])
                wt_s = Psm.tile([128, 64], f32, tag="wtsb")
                nc.vector.tensor_copy(wt_s[:], wtp[:])
                nc.tensor.matmul(cxp[:], wt_s[:], xs_sb[sc][:],
                                 start=(sc == 0), stop=(sc == nchunk - 1))
            cx_sb = Psm.tile([64, H], f32, tag="cxsb")
            nc.vector.tensor_copy(cx_sb[:], cxp[:])
            nc.sync.dma_start(ctx[hb * 64:(hb + 1) * 64, :], cx_sb[:])

        # interleave units of slot pairs so the big slot's dense tanh work
        # fills the small slot's DVE-bound gaps; end on the cheapest slot
        # (0) for a short tail. Early-KT slots (1, 0) lead each pair so
        # work starts before the full KT/QT tiles are ready.
        for first, second in ((1, 2), (3, 0)):
            scs = {first: preamble(first), second: preamble(second)}
            order = []
            for g in range(64 // G):
                for c in range(2):
                    order += [(first, g, c), (second, g, c)]
            # put the pair-closing units so 'second' finishes last
            for hb, g, c in order:
                unit(hb, g, c, scs[hb])
                if hb == first and g == 64 // G - 1 and c == 1:
                    postamble(first, scs[first])
            postamble(second, scs[second])

    nc.compile()
    return nc


def get_nc():
    if "nc" not in _CACHE:
        _CACHE["nc"] = _build_nc()
    return _CACHE["nc"]


def _make_vsel(v):
    # vsel[k, (c*32+col)*32 + m] = v[c*128+k] if m == col else 0
    out = np.zeros((128, 2 * 32 * 32), np.float32)
    for c in range(2):
        for col in range(32):
            out[:, (c * 32 + col) * 32 + col] = v[c * 128:(c + 1) * 128]
    return out.astype(ml_dtypes.bfloat16)


def _make_mask(parity):
    m = np.full((4 * 64, T), -1e30, np.float32)
    for i, (t0, _) in enumerate(HBS[parity]):
        for r in range(64):
            m[i * 64 + r, :t0 + r + 1] = 0.0
    return m


def make_in_maps(rnn_outputs, W1, W2, v):
    X = np.asarray(rnn_outputs, np.float32)
    W1 = np.asarray(W1, np.float32)
    W2 = np.asarray(W2, np.float32)
    v = np.asarray(v, np.float32)
    vsel_np = _make_vsel(v)
    ident_np = np.eye(64, dtype=np.float32)
    masks = {p: _make_mask(p) for p in (0, 1)}
    w1t_np = np.ascontiguousarray(W1.T)
    w2t_np = np.ascontiguousarray(W2.T)
    in_maps = []
    for core in range(NCORES):
        b, p = core // 2, core % 2
        t_rows = np.concatenate([np.arange(t0, t0 + 64) for t0, _ in HBS[p]])
        xb = np.ascontiguousarray(X[b])
        in_maps.append({
            "xqt": np.ascontiguousarray(xb[t_rows].T),
            "xkt": np.ascontiguousarray(xb.T),
            "xs": xb,
            "w1t": w1t_np,
            "w2t": w2t_np,
            "vsel": vsel_np,
            "msk": masks[p],
            "ident": ident_np,
        })
    return in_maps


def assemble(results):
    context = np.zeros((B, T, H), np.float32)
    weights = np.zeros((B, T, T), np.float32)
    for core in range(NCORES):
        b, p = core // 2, core % 2
        w_ = np.asarray(results[core]["wts"]).reshape(4, 64, T)
        c_ = np.asarray(results[core]["ctx"]).reshape(4, 64, H)
        for i, (t0, S) in enumerate(HBS[p]):
            weights[b, t0:t0 + 64, :S] = w_[i, :, :S]
            context[b, t0:t0 + 64] = c_[i]
    return context, weights


def kernel(rnn_outputs, W1, W2, v):
    from concourse.bass_utils import run_bass_kernel_spmd

    nc = get_nc()
    in_maps = make_in_maps(rnn_outputs, W1, W2, v)
    res = run_bass_kernel_spmd(nc, in_maps, list(range(NCORES))).results
    return assemble(res)


# revision 19
# speedup vs baseline: 1.1604x; 1.0336x over previous
"""Bahdanau additive self-attention (causal) on 8 Trainium2 NeuronCores.

reference math (B=4, T=512, H=256):
  q = x @ W1.T ; k = x @ W2.T
  scores[b,t,s] = sum_h v[h] * tanh(q[b,t,h] + k[b,s,h])   (causal: s <= t)
  weights = softmax(scores, axis=-1) ; context = weights @ x
returns (context, weights)

Sharding: 2 cores per batch. Each core handles 4 half-blocks of 64 query
rows chosen so every core's causal s-extents are exactly {128,256,384,512}
-> one identical SPMD program; only the per-core host-side gather/scatter
and the additive causal-mask input differ per core.

Per-core kernel layout: tanh input is built in (h=partition, s=free)
layout so the q[t]+k[s] add is a per-partition tensor_scalar on DVE and
the sum_h v*tanh() reduction is a matmul with a column-selector matrix
built from v (accumulating score rows into PSUM via 32-column tiling).
"""

import numpy as np
import ml_dtypes

B, T, H = 4, 512, 256
NCORES = 8
G = 16  # t-rows per fat qk tile

# per-parity half-blocks: (t0, padded causal extent S)
HBS = {
    0: [(0, 128), (128, 256), (320, 384), (448, 512)],
    1: [(64, 128), (192, 256), (256, 384), (384, 512)],
}
EXTENTS = [128, 256, 384, 512]

_CACHE = {}


def _build_nc():
    from contextlib import ExitStack

    import concourse.bass as bass
    import concourse.tile as tile
    from concourse import bacc, mybir

    f32 = mybir.dt.float32
    bf16 = mybir.dt.bfloat16
    AF = mybir.ActivationFunctionType
    AX = mybir.AxisListType
    ALU = mybir.AluOpType
    PSUM = bass.MemorySpace.PSUM

    nc = bacc.Bacc("TRN2", target_bir_lowering=False, debug=False, num_devices=NCORES)

    xqt = nc.dram_tensor("xqt", [H, 256], f32, kind="ExternalInput").ap()
    xkt = nc.dram_tensor("xkt", [H, T], f32, kind="ExternalInput").ap()
    xs = nc.dram_tensor("xs", [T, H], f32, kind="ExternalInput").ap()
    w1t = nc.dram_tensor("w1t", [H, H], f32, kind="ExternalInput").ap()
    w2t = nc.dram_tensor("w2t", [H, H], f32, kind="ExternalInput").ap()
    vsel = nc.dram_tensor("vsel", [128, 2 * 32 * 32], bf16, kind="ExternalInput").ap()
    msk = nc.dram_tensor("msk", [4 * 64, T], f32, kind="ExternalInput").ap()
    ident = nc.dram_tensor("ident", [64, 64], f32, kind="ExternalInput").ap()
    wts = nc.dram_tensor("wts", [4 * 64, T], f32, kind="ExternalOutput").ap()
    ctx = nc.dram_tensor("ctx", [256, H], f32, kind="ExternalOutput").ap()

    with tile.TileContext(nc) as tc, ExitStack() as es:
        P1 = es.enter_context(tc.tile_pool(name="persist", bufs=1))
        Pqk = es.enter_context(tc.tile_pool(name="qk", bufs=4))
        Pth = es.enter_context(tc.tile_pool(name="th", bufs=4))
        Psm = es.enter_context(tc.tile_pool(name="sm", bufs=2))
        Pset = es.enter_context(tc.tile_pool(name="pset", bufs=2, space=PSUM))
        Psc = es.enter_context(tc.tile_pool(name="psc", bufs=2, space=PSUM))
        Pwt = es.enter_context(tc.tile_pool(name="pwt", bufs=2, space=PSUM))
        Pcx = es.enter_context(tc.tile_pool(name="pcx", bufs=1, space=PSUM))

        # ---- persistent loads: split into ~64KB pieces across many DMA
        #      queues (single-queue BW is ~25GB/s), critical path first ----
        def load(dram_ap, shape, dtype, tag, eng=None, pieces=1):
            t_ = P1.tile(shape, dtype, tag=tag)
            w = shape[1] // pieces
            for pc in range(pieces):
                (eng or nc.sync).dma_start(
                    t_[:, pc * w:(pc + 1) * w],
                    dram_ap[:, pc * w:(pc + 1) * w])
            return t_

        # ACT table warmup first, fed by a memset tile (no DMA dependency)
        warm = Psm.tile([1, 1], f32, tag="warm")
        nc.vector.memset(warm[:], 0.25)
        warm2 = Psm.tile([1, 1], f32, tag="warm2")
        nc.scalar.activation(warm2[:], warm[:], AF.Tanh)

        # slots 0/1 (processed in the first wave) only need k-cols [0:256]
        # and q-cols [0:128]; load those pieces first, spread over several
        # queue engines so dispatch parallelizes, and build small early
        # KT/QT tiles so tanh work starts sooner.
        xktA = [load(xkt[k * 128:(k + 1) * 128, 0:256], [128, 256], f32,
                     f"xktA{k}", pieces=2,
                     eng=(nc.sync if k == 0 else nc.scalar))
                for k in range(2)]
        w2t_sb = [load(w2t[k * 128:(k + 1) * 128, :], [128, H], f32, f"w2t{k}")
                  for k in range(2)]
        xqtA = [load(xqt[k * 128:(k + 1) * 128, 0:128], [128, 128], f32,
                     f"xqtA{k}", eng=nc.scalar) for k in range(2)]
        w1t_sb = [load(w1t[k * 128:(k + 1) * 128, :], [128, H], f32, f"w1t{k}",
                       eng=nc.scalar) for k in range(2)]
        vsel_p = [load(vsel[:, (2 * p) * 512:(2 * p + 2) * 512],
                       [128, 1024], bf16, f"vselp{p}",
                       eng=(nc.sync if p == 0 else nc.scalar))
                  for p in range(2)]
        xkt_sb = [load(xkt[k * 128:(k + 1) * 128, :], [128, T], f32, f"xkt{k}",
                       eng=nc.gpsimd, pieces=2) for k in range(2)]
        xqt_sb = [load(xqt[k * 128:(k + 1) * 128, :], [128, 256], f32,
                       f"xqt{k}", eng=nc.gpsimd) for k in range(2)]
        ident_sb = load(ident[:, :], [64, 64], f32, "ident", eng=nc.gpsimd)
        xs_sb = [load(xs[s * 128:(s + 1) * 128, :], [128, H], f32, f"xs{s}",
                      eng=nc.gpsimd) for s in range(4)]
        msk_sb = [load(msk[i * 64:(i + 1) * 64, :], [64, T], f32, f"msk{i}",
                       eng=nc.gpsimd) for i in range(4)]

        # ---- KT / QT:   kt[h, s] = k[s, h],  qt[h, t] = q[t_rows[t], h] ----
        kt_early, qt_early = [None, None], [None, None]
        for m in range(2):
            ktp = Pset.tile([128, 256], f32, tag="setup")
            for k in range(2):
                nc.tensor.matmul(ktp[:], w2t_sb[k][:, m * 128:(m + 1) * 128],
                                 xktA[k][:], start=(k == 0), stop=(k == 1))
            k_ = P1.tile([128, 256], bf16, tag=f"kte{m}")
            nc.vector.tensor_copy(k_[:], ktp[:])
            kt_early[m] = k_
            qtp = Pset.tile([128, 128], f32, tag="setup")
            for k in range(2):
                nc.tensor.matmul(qtp[:], w1t_sb[k][:, m * 128:(m + 1) * 128],
                                 xqtA[k][:], start=(k == 0), stop=(k == 1))
            q_ = P1.tile([128, 128], f32, tag=f"qte{m}")
            nc.vector.tensor_copy(q_[:], qtp[:])
            qt_early[m] = q_
        qt_sb, kt_sb = [None, None], [None, None]
        for m in range(2):
            ktp = Pset.tile([128, T], f32, tag="setup")
            for k in range(2):
                nc.tensor.matmul(ktp[:], w2t_sb[k][:, m * 128:(m + 1) * 128],
                                 xkt_sb[k][:], start=(k == 0), stop=(k == 1))
            k_ = P1.tile([128, T], bf16, tag=f"kt{m}")
            nc.vector.tensor_copy(k_[:], ktp[:])
            kt_sb[m] = k_
            qtp = Pset.tile([128, 256], f32, tag="setup")
            for k in range(2):
                nc.tensor.matmul(qtp[:], w1t_sb[k][:, m * 128:(m + 1) * 128],
                                 xqt_sb[k][:], start=(k == 0), stop=(k == 1))
            q_ = P1.tile([128, 256], f32, tag=f"qt{m}")
            nc.vector.tensor_copy(q_[:], qtp[:])
            qt_sb[m] = q_

        # ---- main ----
        def preamble(hb):
            S = EXTENTS[hb]
            sc_psum = Psc.tile([64, S], f32, tag="scores")
            # ragged extents leave [E_r, S) unwritten by the MMs; memset so
            # the additive causal mask lands on zeros, not stale PSUM
            nc.vector.memset(sc_psum[:], 0.0)
            return sc_psum

        def unit(hb, g, c, sc_psum):
            """One (group, h-chunk): 32 q+k adds, one tanh, 32 score MMs.
            Group g covers 16 t-rows from each 32-row strip so the MM sweep
            can alternate col-groups (LDWEIGHTS of one strip overlaps the
            in-flight MATMUL of the other)."""
            S = EXTENTS[hb]
            kt_use = kt_early if hb in (0, 1) else kt_sb
            qt_use = qt_early if hb in (0, 1) else qt_sb
            half_g = G // 2
            tls = [g * half_g + (i % half_g) + 32 * (i // half_g)
                   for i in range(G)]
            ext = [S - 63 + tl for tl in tls]  # exact causal extent
            offs = [0]
            for e in ext:
                offs.append(offs[-1] + e)
            fd = offs[-1]
            qk = Pqk.tile([128, fd], bf16, tag="qk")
            for i in range(G):
                tcol = hb * 64 + tls[i]
                nc.vector.tensor_scalar_add(
                    qk[:, offs[i]:offs[i] + ext[i]],
                    kt_use[c][:, :ext[i]],
                    qt_use[c][:, tcol:tcol + 1])
            th = Pth.tile([128, fd], bf16, tag="th")
            nc.scalar.activation(th[:], qk[:], AF.Tanh)
            n_g = 64 // G
            for ii in range(G // 2):
                for half in (0, 1):
                    i = ii + (G // 2) * half
                    tl = tls[i]
                    j, col = tl // 32, tl % 32
                    # no start=True anywhere: the DVE memset is the zero
                    # base and every MM accumulates (first touch of a
                    # pending-zero byte overwrites with its product = 0+x)
                    nc.tensor.matmul(
                        sc_psum[32 * j:32 * j + 32, 0:ext[i]],
                        vsel_p[c][:, col * 32:col * 32 + 32],
                        th[:, offs[i]:offs[i] + ext[i]],
                        start=False,
                        stop=(g == n_g - 1 and c == 1 and ii == G // 2 - 1),
                        tile_position=(0, 32 * j),
                        skip_group_check=True)

        def postamble(hb, sc_psum):
            S = EXTENTS[hb]
            # mask + softmax
            nc.vector.tensor_add(sc_psum[:], sc_psum[:], msk_sb[hb][:, :S])
            nmax = Psm.tile([64, 1], f32, tag="nmax")
            nc.vector.tensor_reduce(nmax[:], sc_psum[:], axis=AX.X,
                                    op=ALU.max, negate=True)
            e_sb = Psm.tile([64, S], f32, tag="esb")
            rsum = Psm.tile([64, 1], f32, tag="rsum")
            nc.scalar.activation(e_sb[:], sc_psum[:], AF.Exp, bias=nmax[:],
                                 accum_out=rsum[:])
            rinv = Psm.tile([64, 1], f32, tag="rinv")
            nc.vector.reciprocal(rinv[:], rsum[:])
            w_sb = Psm.tile([64, S], f32, tag="wsb")
            nc.vector.tensor_scalar_mul(w_sb[:], e_sb[:], rinv[:])
            nc.sync.dma_start(wts[hb * 64:(hb + 1) * 64, :S], w_sb[:])

            # context = weights @ x  (transpose weights chunkwise on PE)
            cxp = Pcx.tile([64, H], f32, tag="cxp")
            nchunk = S // 128
            for sc in range(nchunk):
                wtp = Pwt.tile([128, 64], f32, tag="wtp")
                nc.tensor.transpose(wtp[:], w_sb[:, sc * 128:(sc + 1) * 128],
                                    ident_sb[:])
                wt_s = Psm.tile([128, 64], f32, tag="wtsb")
                nc.vector.tensor_copy(wt_s[:], wtp[:])
                nc.tensor.matmul(cxp[:], wt_s[:], xs_sb[sc][:],
                                 start=(sc == 0), stop=(sc == nchunk - 1))
            cx_sb = Psm.tile([64, H], f32, tag="cxsb")
            nc.vector.tensor_copy(cx_sb[:], cxp[:])
            nc.sync.dma_start(ctx[hb * 64:(hb + 1) * 64, :], cx_sb[:])

        # interleave units of slot pairs so the big slot's dense tanh work
        # fills the small slot's DVE-bound gaps; end on the cheapest slot
        # (0) for a short tail. Early-KT slots (1, 0) lead each pair so
        # work starts before the full KT/QT tiles are ready.
        for first, second in ((1, 2), (3, 0)):
            scs = {first: preamble(first), second: preamble(second)}
            uf = [(first, g, c) for g in range(64 // G) for c in range(2)]
            us = [(second, g, c) for g in range(64 // G) for c in range(2)]
            # front-load `first`: two of its units lead (covers `second`'s
            # later KT dependency), and it finishes two units before the
            # end so its postamble/MM-drain overlaps `second`'s last units
            order = [uf[0], uf[1], us[0]]
            fi, si = 2, 1
            while fi < len(uf) or si < len(us):
                if fi < len(uf):
                    order.append(uf[fi])
                    fi += 1
                if si < len(us) and (len(uf) - fi) <= (len(us) - si):
                    order.append(us[si])
                    si += 1
            for hb, g, c in order:
                unit(hb, g, c, scs[hb])
                if (hb, g, c) == uf[-1]:
                    postamble(first, scs[first])
            postamble(second, scs[second])

    nc.compile()
    return nc


def get_nc():
    if "nc" not in _CACHE:
        _CACHE["nc"] = _build_nc()
    return _CACHE["nc"]


def _make_vsel(v):
    # vsel[k, (c*32+col)*32 + m] = v[c*128+k] if m == col else 0
    out = np.zeros((128, 2 * 32 * 32), np.float32)
    for c in range(2):
        for col in range(32):
            out[:, (c * 32 + col) * 32 + col] = v[c * 128:(c + 1) * 128]
    return out.astype(ml_dtypes.bfloat16)


def _make_mask(parity):
    m = np.full((4 * 64, T), -1e30, np.float32)
    for i, (t0, _) in enumerate(HBS[parity]):
        for r in range(64):
            m[i * 64 + r, :t0 + r + 1] = 0.0
    return m


def make_in_maps(rnn_outputs, W1, W2, v):
    X = np.asarray(rnn_outputs, np.float32)
    W1 = np.asarray(W1, np.float32)
    W2 = np.asarray(W2, np.float32)
    v = np.asarray(v, np.float32)
    vsel_np = _make_vsel(v)
    ident_np = np.eye(64, dtype=np.float32)
    masks = {p: _make_mask(p) for p in (0, 1)}
    w1t_np = np.ascontiguousarray(W1.T)
    w2t_np = np.ascontiguousarray(W2.T)
    in_maps = []
    for core in range(NCORES):
        b, p = core // 2, core % 2
        t_rows = np.concatenate([np.arange(t0, t0 + 64) for t0, _ in HBS[p]])
        xb = np.ascontiguousarray(X[b])
        in_maps.append({
            "xqt": np.ascontiguousarray(xb[t_rows].T),
            "xkt": np.ascontiguousarray(xb.T),
            "xs": xb,
            "w1t": w1t_np,
            "w2t": w2t_np,
            "vsel": vsel_np,
            "msk": masks[p],
            "ident": ident_np,
        })
    return in_maps


def assemble(results):
    context = np.zeros((B, T, H), np.float32)
    weights = np.zeros((B, T, T), np.float32)
    for core in range(NCORES):
        b, p = core // 2, core % 2
        w_ = np.asarray(results[core]["wts"]).reshape(4, 64, T)
        c_ = np.asarray(results[core]["ctx"]).reshape(4, 64, H)
        for i, (t0, S) in enumerate(HBS[p]):
            weights[b, t0:t0 + 64, :S] = w_[i, :, :S]
            context[b, t0:t0 + 64] = c_[i]
    return context, weights


def kernel(rnn_outputs, W1, W2, v):
    from concourse.bass_utils import run_bass_kernel_spmd

    nc = get_nc()
    in_maps = make_in_maps(rnn_outputs, W1, W2, v)
    res = run_bass_kernel_spmd(nc, in_maps, list(range(NCORES))).results
    return assemble(res)
